# revision 7
# baseline (speedup 1.0000x reference)
"""Trainium2 Bass kernel for nn_Block_41077067219413.

Reference computation (B=2048, D=dim_in=4096, J=dim_out=4096):
    xf = x.astype(f32)                 # (B, D) in {0,1}
    mf = masks.astype(f32)             # (D, J) in {0,1}
    sums = xf @ mf + (1-xf) @ (1-mf)   # XNOR popcount over D
    out  = sums > thresholds[None, :]  # (B, J) bool

Identity used here: with x' = 2x-1 in {-1,+1} and m in {0,1},
    A[b,j]   = sum_k x'[b,k] * m[k,j]
    sums     = A + D - rowsum_x[b]
    out      = A - th[j] > rowsum_x[b] - D

One fp8 GEMM per core (batch-sharded 8 ways).  The thresholds are folded
into the GEMM as 4 extra contraction rows carrying base-8 digits of th
with weights (-1,-8,-64,-64) (th = d0 + 8*d1 + 64*d2 + 512*d3; the d3 row
stores 8*d3), so PSUM holds A - th directly.  The epilogue is a single
per-partition-scalar is_gt producing uint8.

Sharding: data-parallel on batch.  masks/thresholds replicated; each core
computes a [256, 4096] output slab; host concatenates.
"""

import numpy as np

B, D, J = 2048, 4096, 4096
NCORES = 8
BL = B // NCORES          # 256 rows per core
P = 128
KT = D // P               # 32 k-tiles
NB = BL // P              # 2 b-tiles per core
JN = 512                  # matmul free-dim tile (one PSUM bank)
JT = J // JN              # 8 j-tiles

USE_DOUBLEROW = True

_cache = {}


def _build():
    import concourse.bacc as bacc
    import concourse.mybir as mybir
    import concourse.tile as tile
    from concourse import masks as masks_lib

    dt = mybir.dt
    f8 = dt.float8e4
    f32 = dt.float32
    AF = mybir.ActivationFunctionType
    ALU = mybir.AluOpType

    nc = bacc.Bacc("TRN2", target_bir_lowering=False, debug=False,
                   num_devices=NCORES)

    x_d = nc.dram_tensor("x", [BL, D], dt.int32, kind="ExternalInput")
    m_d = nc.dram_tensor("masks", [D, J], dt.uint8, kind="ExternalInput")
    th_d = nc.dram_tensor("th", [1, J], dt.int32, kind="ExternalInput")
    cst8_d = nc.dram_tensor("cst8", [4, P], f8, kind="ExternalInput")
    csts_d = nc.dram_tensor("csts", [4, 2], dt.int32, kind="ExternalInput")
    o_d = nc.dram_tensor("out", [BL, J], dt.uint8, kind="ExternalOutput")

    with tile.TileContext(nc) as tc:
        with (
            tc.tile_pool(name="const", bufs=1) as constp,
            tc.tile_pool(name="mask", bufs=1) as maskp,
            tc.tile_pool(name="xt", bufs=1) as xtp,
            tc.tile_pool(name="ob", bufs=2) as obufp,
        ):
            # ---- masks: DMA-cast uint8 -> fp8 {0.0, 1.0}, resident in SBUF.
            # DoubleRow layout: 16 pair-tiles [128, 2, J]; normal: 32 [128, J].
            mask_tiles = []
            if USE_DOUBLEROW:
                for kp in range(KT // 2):
                    mt = maskp.tile([P, 2, J], f8, name=f"mk{kp}", tag=f"mk{kp}")
                    src = m_d[kp * 2 * P:(kp + 1) * 2 * P, :].rearrange(
                        "(ko ki) j -> ki ko j", ki=P)
                    nc.gpsimd.dma_start(mt[:], src)
                    mask_tiles.append(mt)
            else:
                for k in range(KT):
                    mt = maskp.tile([P, J], f8, name=f"mk{k}", tag=f"mk{k}")
                    nc.gpsimd.dma_start(mt[:], m_d[k * P:(k + 1) * P, :])
                    mask_tiles.append(mt)

            # ---- constants
            identity8 = constp.tile([P, P], f8)
            masks_lib.make_identity(nc, identity8[:])

            # digit-row stationary weights + shift/and tables (host consts)
            wstar = constp.tile([4, P], f8)
            nc.sync.dma_start(wstar[:], cst8_d[:])
            shiftands = constp.tile([4, 2], dt.int32)
            nc.sync.dma_start(shiftands[:], csts_d[:])
            shifts = shiftands[:, 0:1]
            ands = shiftands[:, 1:2]

            neg1 = constp.tile([P, 1], f32)
            nc.gpsimd.memset(neg1[:], -1.0)

            # per-b epilogue scalars
            rxe = constp.tile([P, NB], f32)

            # ---- thresholds -> base-8 digit rows [4, J] fp8
            with tc.tile_pool(name="thp", bufs=1) as thp:
                th4 = thp.tile([4, J], dt.int32)
                for i in range(4):
                    nc.sync.dma_start(th4[i:i + 1, :], th_d[:])
                dig_i = thp.tile([4, J], dt.int32)
                nc.vector.tensor_scalar(
                    dig_i[:], th4[:], shifts, ands,
                    op0=ALU.arith_shift_right, op1=ALU.bitwise_and)
                dig8 = constp.tile([4, J], f8)
                nc.vector.tensor_copy(dig8[:], dig_i[:])

            # ---- x prep: int32 -> fp8 {-1,+1} [b, k] + rowsum; PE transpose
            # into xT [128, KT, 256]  (dim1 = k-tile index, dim2 = b columns)
            xT = xtp.tile([P, KT, NB * P], f8)
            with (
                tc.tile_pool(name="xio", bufs=2) as xiop,
                tc.tile_pool(name="xpm", bufs=2) as xpmp,
                tc.tile_pool(name="pstp", bufs=4, space="PSUM") as pstp,
            ):
                for b in range(NB):
                    xi = xiop.tile([P, D], dt.int32, tag="xi")
                    nc.sync.dma_start(xi[:], x_d[b * P:(b + 1) * P, :])
                    xpm = xpmp.tile([P, D], f8, tag="xpm")
                    rxa = xpmp.tile([P, 1], f32, tag="rxa")
                    nc.scalar.activation(
                        xpm[:], xi[:], AF.Identity,
                        bias=neg1[:], scale=2.0, accum_out=rxa[:])
                    # rxe_b = 0.5*accum - 2048  (= rowsum_x - 4096)
                    nc.vector.tensor_scalar(
                        rxe[:, b:b + 1], rxa[:], 0.5, -2048.0,
                        op0=ALU.mult, op1=ALU.add)
                    for pp in range(KT // 2):
                        pst = pstp.tile([P, 2, P, 2], f8, tag="pst")
                        for q in range(2):
                            k = 2 * pp + q
                            nc.tensor.transpose(
                                pst[:, q, :, 0],
                                xpm[:, k * P:(k + 1) * P],
                                identity8[:])
                        nc.vector.tensor_copy(
                            xT[:, 2 * pp:2 * pp + 2, b * P:(b + 1) * P],
                            pst[:, :, :, 0])

            # ---- main GEMM + fused threshold + epilogue
            with tc.tile_pool(name="psacc", bufs=1, space="PSUM") as psacc:
                for b in range(NB):
                    ob = obufp.tile([P, J], dt.uint8, tag="ob")
                    ps = [psacc.tile([P, JN], f32, name=f"acc{j}",
                                     tag=f"acc{j}")
                          for j in range(JT)]
                    if USE_DOUBLEROW:
                        for kp in range(KT // 2):
                            w = xT[:, 2 * kp:2 * kp + 2, b * P:(b + 1) * P]
                            mt = mask_tiles[kp]
                            for j in range(JT):
                                nc.tensor.matmul(
                                    ps[j][:], w,
                                    mt[:, :, j * JN:(j + 1) * JN],
                                    start=(kp == 0), stop=False,
                                    perf_mode=mybir.MatmulPerfMode.DoubleRow)
                    else:
                        for k in range(KT):
                            w = xT[:, k, b * P:(b + 1) * P]
                            mt = mask_tiles[k]
                            for j in range(JT):
                                nc.tensor.matmul(
                                    ps[j][:], w,
                                    mt[:, j * JN:(j + 1) * JN],
                                    start=(k == 0), stop=False)
                    for j in range(JT):
                        # fold thresholds: psum -= th (base-8 digit rows)
                        nc.tensor.matmul(
                            ps[j][:], wstar[:],
                            dig8[:, j * JN:(j + 1) * JN],
                            start=False, stop=True)
                        # out = (A - th > rowsum_x - 4096)
                        nc.vector.tensor_scalar(
                            ob[:, j * JN:(j + 1) * JN], ps[j][:],
                            rxe[:, b:b + 1], None, op0=ALU.is_gt)
                    nc.sync.dma_start(o_d[b * P:(b + 1) * P, :], ob[:])

    nc.compile()
    return nc


def _get_nc():
    if "nc" not in _cache:
        _cache["nc"] = _build()
    return _cache["nc"]


def _cst8():
    import ml_dtypes
    w = np.array([-1.0, -8.0, -64.0, -64.0], dtype=np.float32)
    return np.broadcast_to(w[:, None], (4, P)).astype(ml_dtypes.float8_e4m3)


def _csts():
    return np.array([[0, 7], [3, 7], [6, 7], [6, 56]], dtype=np.int32)


def run(x, masks, thresholds, trace=False):
    """Run the SPMD kernel on 8 cores. Returns (out_bool, BassKernelResults)."""
    from concourse.bass_utils import run_bass_kernel_spmd

    nc = _get_nc()
    m_u8 = np.ascontiguousarray(masks.view(np.uint8))
    th = np.ascontiguousarray(thresholds.reshape(1, J).astype(np.int32))
    in_maps = []
    for c in range(NCORES):
        in_maps.append({
            "x": np.ascontiguousarray(x[c * BL:(c + 1) * BL, :]),
            "masks": m_u8,
            "th": th,
            "cst8": _cst8(),
            "csts": _csts(),
        })
    res = run_bass_kernel_spmd(nc, in_maps, core_ids=list(range(NCORES)),
                               trace=trace)
    out = np.concatenate([r["out"] for r in res.results], axis=0)
    return out.view(np.bool_), res


def kernel(x, masks, thresholds):
    x = np.asarray(x)
    masks = np.asarray(masks)
    thresholds = np.asarray(thresholds)
    out, _ = run(x, masks, thresholds, trace=False)
    return out


# revision 9
# speedup vs baseline: 1.1918x; 1.1918x over previous
"""Trainium2 Bass kernel for nn_Block_41077067219413.

Reference computation (B=2048, D=dim_in=4096, J=dim_out=4096):
    xf = x.astype(f32)                 # (B, D) in {0,1}
    mf = masks.astype(f32)             # (D, J) in {0,1}
    sums = xf @ mf + (1-xf) @ (1-mf)   # XNOR popcount over D
    out  = sums > thresholds[None, :]  # (B, J) bool

Identity used: with x' = 2x-1 in {-1,+1} and m in {0,1},
    A[b,j] = sum_k x'[b,k] * m[k,j]
    sums   = A + D - rowsum_x[b]
    out    = A - th[j] > rowsum_x[b] - D

One fp8 GEMM per core (batch-sharded 8 ways).  masks bytes {0,1} are DMA'd
raw and bitcast to fp8e4, where 0x01 is the subnormal eps=2^-9 — the GEMM
computes eps*A exactly (integers scaled by eps are exact in fp32).
Thresholds are folded into the GEMM as 4 extra contraction rows carrying
base-8 digits of th with eps-scaled weights, so PSUM = eps*(A - th).
Epilogue: single per-partition-scalar is_gt vs eps*(rowsum_x - D) -> uint8.
"""

import numpy as np

B, D, J = 2048, 4096, 4096
NCORES = 8
BL = B // NCORES          # 256 rows per core
P = 128
KT = D // P               # 32 k-tiles
NB = BL // P              # 2 b-tiles per core
JN = 512                  # matmul free-dim tile (one PSUM bank)
JT = J // JN              # 8 j-tiles

_cache = {}


def _build():
    import concourse.bacc as bacc
    import concourse.mybir as mybir
    import concourse.tile as tile

    dt = mybir.dt
    f8 = dt.float8e4
    f32 = dt.float32
    AF = mybir.ActivationFunctionType
    ALU = mybir.AluOpType
    DR = mybir.MatmulPerfMode.DoubleRow

    nc = bacc.Bacc("TRN2", target_bir_lowering=False, debug=False,
                   num_devices=NCORES)

    x_d = nc.dram_tensor("x", [BL, D], dt.int32, kind="ExternalInput")
    m_d = nc.dram_tensor("masks", [D, J], dt.uint8, kind="ExternalInput")
    th_d = nc.dram_tensor("th", [1, J], dt.int32, kind="ExternalInput")
    cst8_d = nc.dram_tensor("cst8", [4, P], f8, kind="ExternalInput")
    ident_d = nc.dram_tensor("ident", [P, P], f8, kind="ExternalInput")
    csts_d = nc.dram_tensor("csts", [4, 2], dt.int32, kind="ExternalInput")
    o_d = nc.dram_tensor("out", [BL, J], dt.uint8, kind="ExternalOutput")

    with tile.TileContext(nc) as tc:
        with (
            tc.tile_pool(name="const", bufs=1) as constp,
            tc.tile_pool(name="mask", bufs=1) as maskp,
            tc.tile_pool(name="xt", bufs=1) as xtp,
            tc.tile_pool(name="ob", bufs=2) as obufp,
        ):
            # ---- const tables via sync DMA (HWDGE; keeps gpsimd queue free)
            wstar = constp.tile([4, P], f8)
            nc.sync.dma_start(wstar[:], cst8_d[:])
            identity8 = constp.tile([P, P], f8)
            nc.sync.dma_start(identity8[:], ident_d[:])
            shiftands = constp.tile([4, 2], dt.int32)
            nc.sync.dma_start(shiftands[:], csts_d[:])

            neg1 = constp.tile([P, 1], f32)
            nc.vector.memset(neg1[:], -1.0)
            rxe = constp.tile([P, NB], f32)
            dig8 = constp.tile([4, J], f8)

            # ---- thresholds -> base-8 digit rows [4, J] fp8 (scoped temps)
            with tc.tile_pool(name="thp", bufs=1) as thp:
                th4 = thp.tile([4, J], dt.int32)
                for i in range(4):
                    nc.sync.dma_start(th4[i:i + 1, :], th_d[:])
                dig_i = thp.tile([4, J], dt.int32)
                nc.vector.tensor_scalar(
                    dig_i[:], th4[:], shiftands[:, 0:1], shiftands[:, 1:2],
                    op0=ALU.arith_shift_right, op1=ALU.bitwise_and)
                nc.vector.tensor_copy(dig8[:], dig_i[:])

            # ---- masks: raw uint8 DMA (bitcast to fp8 at matmul)
            mask_tiles = []
            for kp in range(KT // 2):
                mt = maskp.tile([P, 2, J], dt.uint8, name=f"mk{kp}",
                                tag=f"mk{kp}")
                src = m_d[kp * 2 * P:(kp + 1) * 2 * P, :].rearrange(
                    "(ko ki) j -> ki ko j", ki=P)
                nc.gpsimd.dma_start(mt[:], src)
                mask_tiles.append(mt)

            # ---- x: int32 -> fp8 {-1,+1} + rowsum; PE-transpose into
            # xT [128, KT, 256] (dim1 = k-tile, dim2 = b columns)
            xT = xtp.tile([P, KT, NB * P], f8)
            with (
                tc.tile_pool(name="xio", bufs=2) as xiop,
                tc.tile_pool(name="xpm", bufs=2) as xpmp,
                tc.tile_pool(name="pstp", bufs=4, space="PSUM") as pstp,
            ):
                for b in range(NB):
                    xi = xiop.tile([P, D], dt.int32, tag="xi", name=f"xi{b}")
                    nc.sync.dma_start(xi[:], x_d[b * P:(b + 1) * P, :])
                    xpm = xpmp.tile([P, D], f8, tag="xpm")
                    rxa = xpmp.tile([P, 1], f32, tag="rxa")
                    nc.scalar.activation(
                        xpm[:], xi[:], AF.Identity,
                        bias=neg1[:], scale=2.0, accum_out=rxa[:])
                    # rxe_b = eps*(rowsum_x - D) = accum/1024 - 4
                    nc.vector.tensor_scalar(
                        rxe[:, b:b + 1], rxa[:], 1.0 / 1024.0, -4.0,
                        op0=ALU.mult, op1=ALU.add)
                    for pp in range(KT // 2):
                        pst = pstp.tile([P, 2, P, 2], f8, tag="pst")
                        for q in range(2):
                            k = 2 * pp + q
                            nc.tensor.transpose(
                                pst[:, q, :, 0],
                                xpm[:, k * P:(k + 1) * P],
                                identity8[:])
                        nc.vector.tensor_copy(
                            xT[:, 2 * pp:2 * pp + 2, b * P:(b + 1) * P],
                            pst[:, :, :, 0])

            # ---- main GEMM + fused threshold + epilogue
            with tc.tile_pool(name="psacc", bufs=1, space="PSUM") as psacc:
                for b in range(NB):
                    ob = obufp.tile([P, J], dt.uint8, tag="ob")
                    ps = [psacc.tile([P, JN], f32, name=f"acc{j}",
                                     tag=f"acc{j}")
                          for j in range(JT)]
                    for kp in range(KT // 2):
                        w = xT[:, 2 * kp:2 * kp + 2, b * P:(b + 1) * P]
                        mt = mask_tiles[kp]
                        for j in range(JT):
                            nc.tensor.matmul(
                                ps[j][:], w,
                                mt[:, :, j * JN:(j + 1) * JN].bitcast(f8),
                                start=(kp == 0), stop=False,
                                perf_mode=DR)
                    for j in range(JT):
                        # psum -= eps*th  (base-8 digit rows)
                        nc.tensor.matmul(
                            ps[j][:], wstar[:],
                            dig8[:, j * JN:(j + 1) * JN],
                            start=False, stop=True)
                        # out = (eps*(A - th) > eps*(rowsum_x - D))
                        nc.vector.tensor_scalar(
                            ob[:, j * JN:(j + 1) * JN], ps[j][:],
                            rxe[:, b:b + 1], None, op0=ALU.is_gt)
                    nc.sync.dma_start(o_d[b * P:(b + 1) * P, :], ob[:])

    nc.compile()
    return nc


def _get_nc():
    if "nc" not in _cache:
        _cache["nc"] = _build()
    return _cache["nc"]


def _cst8():
    import ml_dtypes
    # eps-scaled digit weights: -eps*8^i per digit row (row 3 holds 8*d3,
    # so its weight is -eps*512/8 = -2^-3)
    w = np.array([-2.0 ** -9, -2.0 ** -6, -2.0 ** -3, -2.0 ** -3],
                 dtype=np.float32)
    return np.broadcast_to(w[:, None], (4, P)).astype(ml_dtypes.float8_e4m3)


def _ident():
    import ml_dtypes
    return np.eye(P, dtype=np.float32).astype(ml_dtypes.float8_e4m3)


def _csts():
    return np.array([[0, 7], [3, 7], [6, 7], [6, 56]], dtype=np.int32)


def run(x, masks, thresholds, trace=False):
    """Run the SPMD kernel on 8 cores. Returns (out_bool, BassKernelResults)."""
    from concourse.bass_utils import run_bass_kernel_spmd

    nc = _get_nc()
    m_u8 = np.ascontiguousarray(masks.view(np.uint8))
    th = np.ascontiguousarray(thresholds.reshape(1, J).astype(np.int32))
    in_maps = []
    for c in range(NCORES):
        in_maps.append({
            "x": np.ascontiguousarray(x[c * BL:(c + 1) * BL, :]),
            "masks": m_u8,
            "th": th,
            "cst8": _cst8(),
            "ident": _ident(),
            "csts": _csts(),
        })
    res = run_bass_kernel_spmd(nc, in_maps, core_ids=list(range(NCORES)),
                               trace=trace)
    out = np.concatenate([r["out"] for r in res.results], axis=0)
    return out.view(np.bool_), res


def kernel(x, masks, thresholds):
    x = np.asarray(x)
    masks = np.asarray(masks)
    thresholds = np.asarray(thresholds)
    out, _ = run(x, masks, thresholds, trace=False)
    return out
